# revision 1
# baseline (speedup 1.0000x reference)
"""Trainium2 Bass kernel for nn_AdversaryLayer_38723425140998.

RNN language-model layer: per step t (S=512 steps, B=256 batch, U=Z_K=256):
    h   = tanh(h_W[zsh_t] + h1_prev @ h_U + h_b)
    f,i = sigmoid(h @ {f,i}_W + b);  c = tanh(h @ c_W + b);  o = sigmoid(h @ o_W + b)
    h1  = h1_prev * f + c * i
    y_t = softmax(tanh((o * h1) @ t_W + t_b) @ y_W + y_b)

Strategy (8 NeuronCores):
  - Data-parallel: batch 256 -> 32 per core; weights replicated.
  - "Transposed space": state kept as h1^T [256 units (2x128 partitions), 32 batch].
    Weight matrices are the matmul stationary operand (bf16, fp32 PSUM accum);
    the moving operand is the narrow state (N=32).
  - Embedding: bf16 table (h_W + h_b) built on device in DRAM; rows gathered via
    indirect DMA, DMA-transposed into a resident SBUF E^T tile [128, 2, S*32].
  - Fused software pipeline: the recurrent chain (hU -> tanh -> f,i,c -> h1) is
    the critical path; the o/t_W/y_W/softmax chain trails behind, processed in
    2-step pairs to halve its instruction count, and fills TensorE gaps.
  - ScalarE (ACT) is the modeled bottleneck: activation instructions are grouped
    (single sigmoid over f|i, pairwise o / tt), the softmax Exp is batched 4
    pairs (8 steps) at a time so the Sigmoid<->Exp LUT-table switch (1283 ns)
    amortizes, and the softmax normalization runs on GpSimd (idle engine).
  - Zero biases (the harness case) are detected at runtime and specialize the
    build: per-(gate,chunk) ACT bias instructions collapse into grouped ones.
  - bf16 everywhere except PSUM accumulation and the softmax (fp32). Verified in
    numpy: output absmax error ~1e-6 against an fp32 reference (outputs ~4e-3).
"""
import os
import sys
from contextlib import ExitStack

for _p in ("/opt/trn_rl_repo", "/root/.axon_site/_ro/trn_rl_repo"):
    if os.path.isdir(_p) and _p not in sys.path:
        sys.path.insert(0, _p)

import numpy as np

import concourse.bass as bass
import concourse.tile as tile
from concourse import bacc, mybir
from concourse.bass import IndirectOffsetOnAxis
from concourse.bass_utils import run_bass_kernel_spmd
from concourse.masks import make_identity

F32 = mybir.dt.float32
BF16 = mybir.dt.bfloat16
I32 = mybir.dt.int32
AF = mybir.ActivationFunctionType

P = 128          # partitions
UC = 2           # unit chunks (256 units / 128)
ZK = 256         # vocab / output classes
U = 256          # hidden units
N_CORES = 8
B_FULL = 256
S_FULL = 512
BL = B_FULL // N_CORES  # 32 batch per core
TB = 4                  # steps per embedding-gather block (TB*BL = 128 rows)
RING = 4                # softmax ring: pairs per flush (8 steps)


def _bcast(ap, nparts):
    """Prepend a stride-0 partition dim: [..shape..] -> [nparts, ..shape..]."""
    return bass.AP(tensor=ap.tensor, offset=ap.offset, ap=[[0, nparts]] + list(ap.ap))


def build_kernel(S=S_FULL, use_bias=False, s_compute=None):
    assert S % (2 * RING) == 0 and S % TB == 0
    NP = S // 2  # number of step pairs
    if s_compute is None:
        s_compute = S
    NPC = s_compute // 2
    nc = bacc.Bacc(None)

    z = nc.dram_tensor("z", [BL, S], I32, kind="ExternalInput")
    h_W = nc.dram_tensor("h_W", [ZK + 1, U], F32, kind="ExternalInput")
    h_U = nc.dram_tensor("h_U", [U, U], F32, kind="ExternalInput")
    f_W = nc.dram_tensor("f_W", [U, U], F32, kind="ExternalInput")
    i_W = nc.dram_tensor("i_W", [U, U], F32, kind="ExternalInput")
    c_W = nc.dram_tensor("c_W", [U, U], F32, kind="ExternalInput")
    o_W = nc.dram_tensor("o_W", [U, U], F32, kind="ExternalInput")
    t_W = nc.dram_tensor("t_W", [U, U], F32, kind="ExternalInput")
    y_W = nc.dram_tensor("y_W", [U, ZK], F32, kind="ExternalInput")
    h_b = nc.dram_tensor("h_b", [U], F32, kind="ExternalInput")
    f_b = nc.dram_tensor("f_b", [U], F32, kind="ExternalInput")
    i_b = nc.dram_tensor("i_b", [U], F32, kind="ExternalInput")
    c_b = nc.dram_tensor("c_b", [U], F32, kind="ExternalInput")
    o_b = nc.dram_tensor("o_b", [U], F32, kind="ExternalInput")
    t_b = nc.dram_tensor("t_b", [U], F32, kind="ExternalInput")
    y_b = nc.dram_tensor("y_b", [ZK], F32, kind="ExternalInput")
    h0 = nc.dram_tensor("h0", [1, U], F32, kind="ExternalInput")
    y = nc.dram_tensor("y", [BL, S, ZK], F32, kind="ExternalOutput")

    with tile.TileContext(nc) as tc, ExitStack() as ctx:
        singles = ctx.enter_context(tc.tile_pool(name="singles", bufs=1))
        dramp = ctx.enter_context(tc.tile_pool(name="dram", bufs=1, space="DRAM"))
        work = ctx.enter_context(tc.tile_pool(name="work", bufs=2))

        # ---- bf16 embedding table (h_W + h_b) in DRAM --------------------
        table = dramp.tile([ZK + 1, U], BF16)
        hb_bc = singles.tile([P, U], F32, tag="hb_bc")
        nc.sync.dma_start(out=hb_bc[:], in_=_bcast(h_b[:], P))
        for r0, rows in ((0, P), (P, P), (2 * P, 1)):
            tmp = work.tile([P, U], F32, tag="tblrow")
            nc.sync.dma_start(out=tmp[:rows], in_=h_W[r0:r0 + rows, :])
            tb16 = work.tile([P, U], BF16, tag="tblrow16")
            nc.vector.tensor_add(out=tb16[:rows], in0=tmp[:rows], in1=hb_bc[:rows])
            nc.sync.dma_start(out=table[r0:r0 + rows, :], in_=tb16[:rows])

        # ---- weights -> bf16 SBUF chunk tiles ----------------------------
        # wX[p, k, m, q]: stationary chunk (k, m) is wX[:, k, m, :] = W[128k+p, 128m+q]
        def load_w(w_dram, name):
            t32 = work.tile([P, UC, U], F32, tag="wtmp")
            nc.sync.dma_start(out=t32[:], in_=w_dram.rearrange("(k p) m -> p k m", p=P))
            t16 = singles.tile([P, UC, UC, P], BF16, tag=name)
            nc.vector.tensor_copy(
                out=t16[:], in_=t32[:].rearrange("p k (m q) -> p k m q", q=P))
            return t16

        wu = load_w(h_U, "wu")
        wf = load_w(f_W, "wf")
        wi = load_w(i_W, "wi")
        wc = load_w(c_W, "wc")
        # pre-double c_W so a single Tanh(scale=0.5) instr serves f, i and c
        nc.vector.tensor_scalar_mul(wc[:], wc[:], 2.0)
        wo = load_w(o_W, "wo")
        wt = load_w(t_W, "wt")
        # y_W used as the moving operand: wy[:, k, :] = y_W[128k+p, :]
        wy32 = work.tile([P, UC, ZK], F32, tag="wtmp")
        nc.sync.dma_start(out=wy32[:], in_=y_W.rearrange("(k p) m -> p k m", p=P))
        wy = singles.tile([P, UC, ZK], BF16, tag="wy")
        nc.vector.tensor_copy(out=wy[:], in_=wy32[:])

        # ---- biases (only loaded/applied when nonzero) -------------------
        if use_bias:
            def load_b(b_dram, name):
                t = singles.tile([P, UC], F32, tag=name)
                nc.sync.dma_start(out=t[:], in_=b_dram.rearrange("(m p) -> p m", p=P))
                return t

            fb = load_b(f_b, "fb")
            ib = load_b(i_b, "ib")
            cb = load_b(c_b, "cb")
            ob = load_b(o_b, "ob")
            tb_ = load_b(t_b, "tb")
            # sigmoid(x+b) = 0.5*(1+tanh((x+b)/2)): pre-halve the sigmoid biases
            for bt in (fb, ib, ob):
                nc.vector.tensor_scalar_mul(bt[:], bt[:], 0.5)
            ybt = singles.tile([2 * BL, ZK], F32, tag="ybt")
            nc.sync.dma_start(out=ybt[:], in_=_bcast(y_b[:], 2 * BL))

        # identity stationary for accumulating E^T into PSUM via TensorE
        ident = singles.tile([P, P], BF16, tag="ident")
        make_identity(nc, ident[:])

        # ---- initial hidden state (h0 broadcast over batch) --------------
        h0t = singles.tile([P, UC, 1], F32, tag="h0t")
        nc.sync.dma_start(out=h0t[:, :, 0], in_=h0.rearrange("one (c p) -> p (c one)", p=P))
        h1i32 = singles.tile([P, UC, BL], F32, tag="h1i32")
        nc.vector.memset(h1i32[:], 0.0)
        for c in range(UC):
            nc.vector.tensor_scalar_add(h1i32[:, c, :], h1i32[:, c, :], h0t[:, c, 0:1])
        h1i = singles.tile([P, UC, BL], BF16, tag="h1i")
        nc.vector.tensor_copy(out=h1i[:], in_=h1i32[:])

        # ---- shifted codes: zsh[b, 0] = 0, zsh[b, t] = z[b, t-1] + 1 -----
        zt = singles.tile([BL, S], I32, tag="zt")
        nc.sync.dma_start(out=zt[:], in_=z[:])
        zsh = singles.tile([BL, S], I32, tag="zsh")
        nc.vector.memset(zsh[:, 0:1], 0)
        nc.vector.tensor_scalar_add(zsh[:, 1:S], zt[:, 0:S - 1], 1)

        # swizzle to gather order: zsw[s*BL + b, blk] = zsh[b, blk*TB + s]
        n_blocks = S // TB
        zsw = singles.tile([P, n_blocks], I32, tag="zsw")
        zsh_v = zsh[:].rearrange("b (blk s) -> b blk s", s=TB)
        for s in range(TB):
            nc.sync.dma_start(out=zsw[s * BL:(s + 1) * BL, :], in_=zsh_v[:, :, s])

        # ---- embedding gather + transpose into resident E^T --------------
        # E[p, c, tok] = table_bf16[zsh[b, t], 128c + p], tok = t*BL + b
        E = singles.tile([P, UC, S * BL], BF16, tag="E")
        gthp = ctx.enter_context(tc.tile_pool(name="gth", bufs=8))
        for blk in range(n_blocks):
            gth = gthp.tile([P, U], BF16)
            nc.gpsimd.indirect_dma_start(
                out=gth[:], out_offset=None, in_=table[:],
                in_offset=IndirectOffsetOnAxis(ap=zsw[:, blk:blk + 1], axis=0))
            for c in range(UC):
                nc.scalar.dma_start(
                    out=E[:, c, blk * P:(blk + 1) * P],
                    in_=gth[:, c * P:(c + 1) * P], transpose=True)

        # ---- pools for the scan ------------------------------------------
        psA = ctx.enter_context(tc.tile_pool(name="psA", bufs=3, space="PSUM"))
        psB = ctx.enter_context(tc.tile_pool(name="psB", bufs=3, space="PSUM"))
        psY = ctx.enter_context(tc.tile_pool(name="psY", bufs=2, space="PSUM"))
        p_preh = ctx.enter_context(tc.tile_pool(name="p_preh", bufs=3))
        p_h = ctx.enter_context(tc.tile_pool(name="p_h", bufs=3))
        p_fic = ctx.enter_context(tc.tile_pool(name="p_fic", bufs=3))
        p_o = ctx.enter_context(tc.tile_pool(name="p_o", bufs=3))
        p_h1 = ctx.enter_context(tc.tile_pool(name="p_h1", bufs=3))
        p_g = ctx.enter_context(tc.tile_pool(name="p_g", bufs=3))
        p_tt = ctx.enter_context(tc.tile_pool(name="p_tt", bufs=3))
        p_yr = ctx.enter_context(tc.tile_pool(name="p_yr", bufs=3))

        h1_prev = h1i
        h_ring = {}     # pair -> h^T ring tile [P, UC, 2, BL]
        g_ring = {}     # pair -> g^T ring tile [P, UC, 2, BL]
        psB_of = {}     # pair -> psum tile [P, 4, 2*BL]: o m0,m1 | t m0,m1
        tt_of = {}      # pair -> tt^T tile [P, UC, 2, BL]
        ynrm_ring = None    # [2*BL, RING, ZK] f32 sbuf

        def flush_ring(last_pj):
            """Store RING pairs of normalized softmax rows."""
            r0 = last_pj - (RING - 1)
            # ynrm[(s, b), r, k] -> y[b, 2*(r0+r) + s, k]
            t0 = 2 * r0
            yv = y[:, t0:t0 + 2 * RING, :].rearrange("b (r s) k -> b s r k", s=2)
            for s in range(2):
                nc.sync.dma_start(
                    out=yv[:, s, :, :], in_=ynrm_ring[s * BL:(s + 1) * BL, :, :])

        for t in range(s_compute + 4):
            if t < s_compute:
                # -- recurrent critical path for step t --
                pj, sl = divmod(t, 2)
                hp = tc.high_priority()
                hp.__enter__()
                pa = psA.tile([P, 8, BL], F32, tag="pa")   # h m0,m1 | f m0,m1 | i m0,m1 | c m0,m1
                for m in range(UC):
                    for k in range(UC):
                        nc.tensor.matmul(
                            out=pa[:, m, :], lhsT=wu[:, k, m, :], rhs=h1_prev[:, k, :],
                            start=(k == 0), stop=False)
                    nc.tensor.matmul(
                        out=pa[:, m, :], lhsT=ident[:],
                        rhs=E[:, m, t * BL:(t + 1) * BL], start=False, stop=True)
                if sl == 0:
                    h_ring[pj] = p_h.tile([P, UC, 2, BL], BF16, name="hr", tag="hr")
                hr = h_ring[pj]
                nc.scalar.activation(out=hr[:, :, sl, :], in_=pa[:, 0:UC, :], func=AF.Tanh)

                for gi, wg in enumerate((wf, wi, wc)):
                    for m in range(UC):
                        for k in range(UC):
                            nc.tensor.matmul(
                                out=pa[:, 2 + 2 * gi + m, :], lhsT=wg[:, k, m, :],
                                rhs=hr[:, k, sl, :], start=(k == 0), stop=(k == UC - 1))

                fic = p_fic.tile([P, 3, UC, BL], BF16, tag="fic")
                if use_bias:
                    for gi, bt in ((0, fb), (1, ib), (2, cb)):
                        for m in range(UC):
                            nc.scalar.activation(
                                out=fic[:, gi, m, :], in_=pa[:, 2 + 2 * gi + m, :],
                                func=AF.Tanh, scale=0.5, bias=bt[:, m:m + 1])
                else:
                    nc.scalar.activation(
                        out=fic[:], in_=pa[:, 2:8, :], func=AF.Tanh, scale=0.5)
                # sigmoid = 0.5*tanh + 0.5 (f and i slots in one op)
                nc.vector.tensor_scalar(
                    fic[:, 0:2, :, :], fic[:, 0:2, :, :], 0.5, 0.5,
                    mybir.AluOpType.mult, mybir.AluOpType.add)

                t1 = p_preh.tile([P, UC, BL], BF16, tag="t1")
                nc.vector.tensor_mul(out=t1[:], in0=fic[:, 0, :, :], in1=h1_prev[:])
                t2 = p_preh.tile([P, UC, BL], BF16, tag="t2")
                nc.vector.tensor_mul(out=t2[:], in0=fic[:, 2, :, :], in1=fic[:, 1, :, :])
                h1 = p_h1.tile([P, UC, BL], BF16, tag="h1")
                nc.vector.tensor_add(out=h1[:], in0=t1[:], in1=t2[:])
                hp.__exit__(None, None, None)

                if sl == 1:
                    # -- o for the completed pair (both steps' h ready) --
                    pb = psB.tile([P, 4, 2 * BL], F32, tag="pb")   # o m0,m1 | t m0,m1
                    psB_of[pj] = pb
                    for m in range(UC):
                        for k in range(UC):
                            nc.tensor.matmul(
                                out=pb[:, m, :], lhsT=wo[:, k, m, :],
                                rhs=hr[:, k, :, :].rearrange("p s b -> p (s b)"),
                                start=(k == 0), stop=(k == UC - 1))
                    osb = p_o.tile([P, UC, 2, BL], BF16, tag="osb")
                    if use_bias:
                        for m in range(UC):
                            nc.scalar.activation(
                                out=osb[:, m, :, :], in_=pb[:, m, :],
                                func=AF.Tanh, scale=0.5, bias=ob[:, m:m + 1])
                    else:
                        nc.scalar.activation(
                            out=osb[:], in_=pb[:, 0:2, :], func=AF.Tanh, scale=0.5)
                    nc.vector.tensor_scalar(
                        osb[:], osb[:], 0.5, 0.5,
                        mybir.AluOpType.mult, mybir.AluOpType.add)
                    # g for both steps of the pair
                    # h1_prev still holds step t-1's h1 here; h1 is step t's.
                    g_ring[pj] = gr = p_g.tile([P, UC, 2, BL], BF16, name="gr", tag="gr")
                    nc.vector.tensor_mul(
                        out=gr[:, :, 0, :], in0=osb[:, :, 0, :], in1=h1_prev[:])
                    nc.vector.tensor_mul(
                        out=gr[:, :, 1, :], in0=osb[:, :, 1, :], in1=h1[:])
                h1_prev = h1

            # -- t_W stage for pair t//2 - 1 (even iterations) --
            if t % 2 == 0 and t >= 2:
                pj1 = t // 2 - 1
                if pj1 < NPC:
                    pb1 = psB_of[pj1]
                    gr1 = g_ring.pop(pj1)
                    del h_ring[pj1]
                    for m in range(UC):
                        for k in range(UC):
                            nc.tensor.matmul(
                                out=pb1[:, 2 + m, :],
                                lhsT=wt[:, k, m, :],
                                rhs=gr1[:, k, :, :].rearrange("p s b -> p (s b)"),
                                start=(k == 0), stop=(k == UC - 1))
                    tt = p_tt.tile([P, UC, 2, BL], BF16, tag="tt")
                    if use_bias:
                        for m in range(UC):
                            nc.scalar.activation(
                                out=tt[:, m, :, :], in_=pb1[:, 2 + m, :],
                                func=AF.Tanh, bias=tb_[:, m:m + 1])
                    else:
                        nc.scalar.activation(
                            out=tt[:], in_=pb1[:, 2:4, :], func=AF.Tanh)
                    tt_of[pj1] = tt

            # -- y stage for pair (t-3)//2 (odd iterations) --
            if t % 2 == 1 and t >= 3:
                pj2 = (t - 3) // 2
                if pj2 < NPC:
                    del psB_of[pj2]
                    tt2 = tt_of.pop(pj2)
                    py = psY.tile([2 * BL, ZK], F32, tag="py")
                    for k in range(UC):
                        nc.tensor.matmul(
                            out=py[:],
                            lhsT=tt2[:, k, :, :].rearrange("p s b -> p (s b)"),
                            rhs=wy[:, k, :], start=(k == 0), stop=(k == UC - 1))
                    r = pj2 % RING
                    if r == 0:
                        ynrm_ring = p_yr.tile(
                            [2 * BL, RING, ZK], F32, name="ynrm", tag="ynrm")
                    yexp = p_yr.tile([2 * BL, ZK], F32, tag="yexp")
                    ysum = p_yr.tile([2 * BL, 1], F32, tag="ysum")
                    if use_bias:
                        ylog = p_yr.tile([2 * BL, ZK], F32, tag="ylog")
                        nc.vector.tensor_add(out=ylog[:], in0=py[:], in1=ybt[:])
                        nc.scalar.activation(
                            out=yexp[:], in_=ylog[:], func=AF.Exp,
                            accum_out=ysum[:])
                    else:
                        nc.scalar.activation(
                            out=yexp[:], in_=py[:], func=AF.Exp, accum_out=ysum[:])
                    yrec = p_yr.tile([2 * BL, 1], F32, tag="yrec")
                    nc.vector.reciprocal(out=yrec[:], in_=ysum[:])
                    # normalize on GpSimd (idle engine; SBUF-only op)
                    nc.gpsimd.tensor_scalar_mul(
                        ynrm_ring[:, r, :], yexp[:], yrec[:, 0:1])
                    if r == RING - 1:
                        flush_ring(pj2)

    nc.finalize()
    return nc


_NC_CACHE = {}


def _get_nc(S, use_bias):
    key = (S, use_bias)
    if key not in _NC_CACHE:
        _NC_CACHE[key] = build_kernel(S, use_bias)
    return _NC_CACHE[key]


def kernel(z, h_W, h_U, h_b, f_W, f_b, i_W, i_b, c_W, c_b,
           o_W, o_b, t_W, t_b, y_W, y_b, h0):
    z = np.ascontiguousarray(np.asarray(z, dtype=np.int32))
    B, S = z.shape
    f32 = lambda a: np.ascontiguousarray(np.asarray(a, dtype=np.float32))
    shared = dict(
        h_W=f32(h_W), h_U=f32(h_U), f_W=f32(f_W), i_W=f32(i_W), c_W=f32(c_W),
        o_W=f32(o_W), t_W=f32(t_W), y_W=f32(y_W), h_b=f32(h_b), f_b=f32(f_b),
        i_b=f32(i_b), c_b=f32(c_b), o_b=f32(o_b), t_b=f32(t_b), y_b=f32(y_b),
        h0=f32(h0).reshape(1, U))
    # h_b always folds into the embedding table for free; the other biases
    # cost extra instructions, so specialize the build when they are all zero.
    use_bias = any(
        np.any(shared[k]) for k in ("f_b", "i_b", "c_b", "o_b", "t_b", "y_b"))
    nc = _get_nc(S, use_bias)
    in_maps = [dict(shared, z=z[c * BL:(c + 1) * BL, :]) for c in range(N_CORES)]
    last_err = None
    for _attempt in range(4):
        try:
            res = run_bass_kernel_spmd(nc, in_maps, list(range(N_CORES)))
            break
        except Exception as e:  # transient NRT/device errors: retry
            last_err = e
            msg = str(e).upper()
            if "UNRECOVERABLE" not in msg and "UNAVAILABLE" not in msg:
                raise
            import time as _time
            _time.sleep(5 * (_attempt + 1))
            try:  # drop cached PJRT state so the retry reconnects cleanly
                import jax
                jax.clear_caches()
            except Exception:
                pass
    else:
        raise last_err
    return np.concatenate(
        [res.results[c]["y"] for c in range(N_CORES)], axis=0).astype(np.float32)


def _numpy_ref(inp):
    z = np.asarray(inp["z"]); B, S = z.shape
    zsh = np.concatenate([np.zeros((B, 1), np.int32), z[:, :-1] + 1], axis=1)
    sig = lambda x: 1 / (1 + np.exp(-x))
    h1 = np.repeat(np.asarray(inp["h0"]).reshape(1, U), B, axis=0).astype(np.float32)
    out = np.zeros((B, S, ZK), np.float32)
    for t in range(S):
        h = np.tanh(inp["h_W"][zsh[:, t]] + h1 @ inp["h_U"] + inp["h_b"])
        f = sig(h @ inp["f_W"] + inp["f_b"]); i = sig(h @ inp["i_W"] + inp["i_b"])
        c = np.tanh(h @ inp["c_W"] + inp["c_b"]); o = sig(h @ inp["o_W"] + inp["o_b"])
        h1 = h1 * f + c * i
        tt = np.tanh((o * h1) @ inp["t_W"] + inp["t_b"])
        lg = tt @ inp["y_W"] + inp["y_b"]
        e = np.exp(lg - lg.max(-1, keepdims=True))
        out[:, t, :] = e / e.sum(-1, keepdims=True)
    return out


if __name__ == "__main__":
    rng = np.random.default_rng(0)
    S = int(sys.argv[1]) if len(sys.argv) > 1 else 16
    zero_bias = len(sys.argv) > 2 and sys.argv[2] == "zero"
    g = lambda shape: (rng.standard_normal(shape) * 0.05).astype(np.float32)
    b = (lambda shape: np.zeros(shape, np.float32)) if zero_bias else g
    inputs = dict(
        z=rng.integers(0, ZK, (B_FULL, S)).astype(np.int32),
        h_W=g((ZK + 1, U)), h_U=g((U, U)), h_b=b((U,)),
        f_W=g((U, U)), f_b=b((U,)),
        i_W=g((U, U)), i_b=b((U,)),
        c_W=g((U, U)), c_b=b((U,)),
        o_W=g((U, U)), o_b=b((U,)),
        t_W=g((U, U)), t_b=b((U,)),
        y_W=g((U, ZK)), y_b=b((ZK,)),
        h0=(np.zeros((1, U), np.float32) if zero_bias
            else (rng.standard_normal((1, U)) * 0.05).astype(np.float32)))
    got = kernel(**inputs)
    exp = _numpy_ref(inputs)
    err = np.abs(got - exp)
    print(f"S={S} zero_bias={zero_bias}  absmax={err.max():.3e}  "
          f"(ref absmax {np.abs(exp).max():.3e})  rel={err.max() / np.abs(exp).max():.3e}")

